# revision 1
# baseline (speedup 1.0000x reference)
"""Trainium2 Bass kernel for nn_DenseProduct (num_factors=2).

Computes, for input x of shape (128, 16, 64, 32) f32:
    out[s, d, b, i*32+j] = x[2s, d, b, i] + x[2s+1, d, b, j]
with output shape (64, 16, 64, 1024) f32.

Sharding: scope axis (dim 0) across 8 NeuronCores — core c gets input
scopes [16c, 16c+16) and produces output scopes [8c, 8c+8), a contiguous
33.5 MB slice of the output per core.

Per-core layout: SBUF partition p = d*8 + b_hi (d in [0,16), b_hi in [0,8),
b = 8*b_hi + b_lo). This makes the input DMA read contiguous 1 KB runs and
the output DMA write one contiguous 4 MB region per scope (32 KB per
partition). The whole outer-sum for one scope is a single DVE tensor_tensor
with stride-0 (broadcast) free dims:
    out[p, (bl, i, j)] = A[p, (bl, i)] + B[p, (bl, j)]
"""

import numpy as np

_S_IN = 128        # total input scopes
_NF = 2            # num_factors (hardcoded)
_S_OUT = _S_IN // _NF
_D = 16
_B = 64
_N = 32
_N_CORES = 8
_SIN_LOC = _S_IN // _N_CORES   # 16 input scopes per core
_S_LOC = _S_OUT // _N_CORES    # 8 output scopes per core
_P = 128
_BH = 8
_BL = 8
_FREE_IN = _BL * _N            # 256
_FREE_OUT = _BL * _N * _N      # 8192

_CACHE = {}
LAST_RESULTS = None  # BassKernelResults of the most recent run (for profiling)


def _build_bass():
    import concourse.bacc as bacc
    import concourse.mybir as mybir
    from concourse.tile import TileContext

    nc = bacc.Bacc("TRN2", target_bir_lowering=False, debug=False,
                   num_devices=_N_CORES)
    x = nc.dram_tensor("x", [_SIN_LOC, _D, _B, _N], mybir.dt.float32,
                       kind="ExternalInput").ap()
    out = nc.dram_tensor("out", [_S_LOC, _D, _B, _N * _N], mybir.dt.float32,
                         kind="ExternalOutput").ap()

    with TileContext(nc) as tc:
        with tc.tile_pool(name="inp", bufs=_S_LOC) as in_pool, \
             tc.tile_pool(name="head", bufs=1) as head_pool, \
             tc.tile_pool(name="outp", bufs=4) as out_pool:
            # x[s_in, d, 8*bh+bl, n] -> partition (d, bh), free (s_in, bl, n)
            xr = x.rearrange("s d (bh bl) n -> (d bh) s (bl n)", bh=_BH)
            # tiny head tile: bl=0 strip of both factors of scope 0, so the
            # very first compute piece (and with it the output DMA stream)
            # starts ~1.5us before the full scope-0 input lands
            ht = head_pool.tile([_P, 2 * _N], mybir.dt.float32)
            nc.sync.dma_start(out=ht[:, :].rearrange("p (s f) -> p s f", s=2),
                              in_=xr[:, 0:2, 0:_N])
            in_tiles = []
            for s in range(_S_LOC):
                # both factors (s_in = 2s, 2s+1) in one DMA -> one wait sem
                t = in_pool.tile([_P, 2 * _FREE_IN], mybir.dt.float32)
                src = xr[:, 2 * s:2 * s + 2]  # (128, 2, 256), s-stride 32768
                dst = t[:, :].rearrange("p (s f) -> p s f", s=2)
                nc.sync.dma_start(out=dst, in_=src)
                in_tiles.append(t)

            ndma = 0
            for s in range(_S_LOC):
                # Pieces are (bl_start, bl_width, i_start, i_width) quarters of
                # the (bl, i) plane. Scope 0 ramps up from a tiny first piece so
                # the first output DMA issues as early as possible; later scopes
                # go out as single 4MB DMAs (large transfers sustain ~425 GB/s;
                # small ones pay ~1us of per-DMA boundary overhead).
                if s == 0:
                    pieces = [(0, 1, 0, 16), (0, 1, 16, 16), (1, 1, 0, _N),
                              (2, 2, 0, _N), (4, 4, 0, _N)]
                elif s in (1, 2, 3, 4):
                    pieces = [(0, 4, 0, _N), (4, 4, 0, _N)]
                else:
                    pieces = [(0, 8, 0, _N)]
                ot = out_pool.tile([_P, _FREE_OUT], mybir.dt.float32)
                dst = out[s].rearrange("d (bh bl) f -> (d bh) (bl f)", bh=_BH)
                for bl0, w, i0, wi in pieces:
                    if s == 0 and bl0 == 0:
                        src_t, off_a, off_b = ht, 0, _N
                    else:
                        src_t, off_a, off_b = in_tiles[s], bl0 * _N, _FREE_IN + bl0 * _N
                    # a: w bl-blocks of wi i-values (i-subrange only for w == 1)
                    a = src_t[:, off_a + i0:off_a + i0 + (w - 1) * _N + wi] \
                        .rearrange("p (bl i) -> p bl i", bl=w)
                    b = src_t[:, off_b:off_b + w * _N] \
                        .rearrange("p (bl j) -> p bl j", bl=w)
                    a4 = a.unsqueeze(3).broadcast_to([_P, w, wi, _N])
                    b4 = b.unsqueeze(2).broadcast_to([_P, w, wi, _N])
                    f0 = bl0 * _N * _N + i0 * _N
                    sz = w * wi * _N
                    osl = ot[:, f0:f0 + sz]
                    o4 = osl.rearrange("p (bl i j) -> p bl i j", bl=w, i=wi)
                    nc.vector.tensor_add(o4, a4, b4)
                    # Two HWDGE rings (SP=sync / ACT=scalar). The first three
                    # (tiny) pieces go on the scalar ring, which is empty while
                    # the input DMAs occupy the sync ring FIFO, so the output
                    # stream starts immediately. Every later DMA strictly
                    # alternates rings — with only one ring active, each DMA's
                    # ~1us completion boundary is exposed; alternation hides it
                    # under the other ring's data stream.
                    if ndma < 3:
                        eng = nc.scalar
                    else:
                        eng = nc.sync if ndma % 2 == 1 else nc.scalar
                    eng.dma_start(out=dst[:, f0:f0 + sz], in_=osl)
                    ndma += 1
    nc.compile()
    return nc


def kernel(x, num_factors):
    global LAST_RESULTS
    from concourse.bass_utils import run_bass_kernel_spmd

    x = np.asarray(x)
    assert x.shape == (_S_IN, _D, _B, _N), x.shape
    assert int(num_factors) == _NF, num_factors
    x = x.astype(np.float32, copy=False)

    if "nc" not in _CACHE:
        _CACHE["nc"] = _build_bass()
    nc = _CACHE["nc"]

    in_maps = [
        {"x": np.ascontiguousarray(x[c * _SIN_LOC:(c + 1) * _SIN_LOC])}
        for c in range(_N_CORES)
    ]
    res = run_bass_kernel_spmd(nc, in_maps, core_ids=list(range(_N_CORES)))
    LAST_RESULTS = res
    out = np.concatenate([res.results[c]["out"] for c in range(_N_CORES)], axis=0)
    return out.reshape(_S_OUT, _D, _B, _N ** _NF)

